# revision 21
# baseline (speedup 1.0000x reference)
"""MultiHeadAttn (post-LN, key-padding mask) Trainium2 Bass kernel, 8 cores.

Problem: h [S=2048, B=4, D=1024] f32; 16 heads x 64; key-padding mask [S, B];
out = LayerNorm(h + MHA(h)).

Sharding: core c handles batch b = c//2 and query half qh = c%2 (1024 query
rows), with all 16 heads and the full 2048-key context for that batch.
KV projections are recomputed by both cores of a batch pair (no collectives).

Per-core device pipeline:
  - All four projections (K^T, Q^T, V, O) and the PV matmul run in fp8e4
    with perf_mode=DoubleRow: operands are [128, 2, free] slot-pair APs so
    each 512-col moving stream contracts 256 rows -> half the PE streams of
    the bf16 version.  Scores stay bf16 (K^T/Q^T psum evacuated to bf16).
  - V proj psum is evacuated by the (otherwise idle in the prephase) ACT
    engine via Copy with per-partition scale z in {0,1} (masked key -> 0);
    a z*(1/16) column is appended per head (col 64), so the DoubleRow PV
    matmul (stationary [z*V | z/16], M=65) accumulates the numerator and,
    in psum row 64, den/16.  The 1/16 keeps the fp8 avt in the normal
    e4m3 range (avt stores 16*attn_vec; the output projection's residual
    STT multiplies by 1/16).
  - Attention per head pair: scores^T [j,i] via row-paired bf16 matmuls
    (two heads in row strips 0-63 / 64-127, i-major issue order), exp via
    ScalarE with scale=1/8 and bias=-2 (softmax-invariant shift keeping
    exp values < 105 so the fp8e4 pt never overflows), written straight to
    fp8 slot j%2 of a [128, 2, 1024] pt tile.  PV consumes jtile PAIRS
    (DoubleRow contraction 256 keys) ~2 j behind the scores/exp front.
  - Normalize (two-stage software pipeline, all off the PE critical path):
    stage 1 evacuates av psum via DVE copies and computes 16/den with a
    32-lane-parallel transpose/strided-reciprocal/transpose; stage 2 -
    deferred to the NEXT pair's end - broadcasts 16/den to 64 rows on
    GPSIMD and multiplies into the fp8 avt slot (head1 via a small
    partition-shift DMA into rows 64-127).
  - Output proj: DoubleRow over e-tile pairs; residual add + LN spread
    over DVE (stt/accum), ScalarE (Square, Sqrt, gamma*rstd) and GPSIMD
    (+beta); 8 psum banks + 3-deep tile pipeline hide the chain.
Next head pair's K/Q projections are interleaved into the attention loop
(borrowing scores-pool PSUM slots) so the PE stays busy under the ACT-bound
softmax stream.
"""
import numpy as np
import ml_dtypes

N_HEAD, D_MODEL, D_HEAD = 16, 1024, 64
SEQ, BSZ = 2048, 4
QLEN = SEQ // 2
SCALE = 1.0 / D_HEAD ** 0.5
EXP_BIAS = -2.0             # softmax-invariant shift; max score 6.65 -> exp<105
LN_EPS = 1e-5
P = 128
NSL = 512                   # matmul moving-operand slab (one PSUM bank fp32)
CT = D_MODEL // P           # 8 contraction tiles
ET = D_MODEL // P           # 8 e-tiles (2 heads each)
JT = SEQ // P               # 16 key tiles
JP = JT // 2                # 8 key-tile pairs (DoubleRow PV)
JS = SEQ // NSL             # 4 key slabs
IS = QLEN // NSL            # 2 query slabs
TQ = QLEN // P              # 8 query-row tiles
HP = N_HEAD // 2            # 8 head pairs
DH1 = D_HEAD + 1            # V columns per head incl. the z (denominator) col
DHP = 80                    # padded per-head V stride (16B-aligned fp8 slots)
AVS = 16.0                  # avt stores AVS*attn_vec (fp8 range use)

_CACHE = {}


def _build():
    from contextlib import ExitStack
    import concourse.bass as bass
    import concourse.mybir as mybir
    import concourse.tile as tile
    from concourse import bacc

    dt = mybir.dt
    f32, bf16, fp8 = dt.float32, dt.bfloat16, dt.float8e4
    AF = mybir.ActivationFunctionType
    ALU = mybir.AluOpType
    DR = mybir.MatmulPerfMode.DoubleRow

    nc = bacc.Bacc(None, target_bir_lowering=False)

    # fp8 host layouts: ht2[p, c, s] = h^T[c*128+p, s]; wq2/wk2[p, e, c, ce] =
    # W[c*128+p, e*128+ce] (e-tile-major for contiguous column loads);
    # wv2/wo2[p, c, e] = W[c*128+p, e].
    ht2 = nc.dram_tensor("ht2", [P, CT, SEQ], fp8, kind="ExternalInput")
    hq = nc.dram_tensor("hq", [QLEN, D_MODEL], f32, kind="ExternalInput")
    wq2 = nc.dram_tensor("wq2", [P, ET, CT, P], fp8, kind="ExternalInput")
    wk2 = nc.dram_tensor("wk2", [P, ET, CT, P], fp8, kind="ExternalInput")
    wv2 = nc.dram_tensor("wv2", [P, CT, D_MODEL], fp8, kind="ExternalInput")
    wo2 = nc.dram_tensor("wo2", [P, CT, D_MODEL], fp8, kind="ExternalInput")
    zt = nc.dram_tensor("zt", [SEQ], f32, kind="ExternalInput")
    gam = nc.dram_tensor("gam", [D_MODEL], f32, kind="ExternalInput")
    bet = nc.dram_tensor("bet", [D_MODEL], f32, kind="ExternalInput")
    out = nc.dram_tensor("out", [QLEN, D_MODEL], f32, kind="ExternalOutput")

    with tile.TileContext(nc) as tc, ExitStack() as ctx:
        persist = ctx.enter_context(tc.tile_pool(name="persist", bufs=1))

        # K/Q tiles die after their pair's scores — roll through 2 slots
        ktq = ctx.enter_context(tc.tile_pool(name="ktq", bufs=2))
        # V in fp8 slot pairs: v2[p, slot, head, 0:64]=z*V, [.., 64]=z/16
        v_sb = [persist.tile([P, 2, N_HEAD, DHP], fp8, name=f"v{t}")
                for t in range(JP)]
        # avt in fp8 slot pairs for the DoubleRow output projection
        avt_sb = [persist.tile([P, 2, QLEN], fp8, name=f"avt{e}")
                  for e in range(ET // 2)]
        z_sb = persist.tile([P, JT], f32, name="zmask")
        ones16 = persist.tile([P, N_HEAD, 1], f32, name="ones16")
        eps_sb = persist.tile([P, 1], f32, name="eps")
        expb_sb = persist.tile([P, 1], f32, name="expb")

        nc.vector.memset(eps_sb, LN_EPS)
        nc.vector.memset(expb_sb, EXP_BIAS)
        nc.vector.memset(ones16, 1.0 / AVS)

        nc.gpsimd.dma_start(out=z_sb,
                            in_=bass.AP(tensor=zt, offset=0, ap=[[1, P], [P, JT]]))

        # ---- phase-3 weights + residual input (prefetched early) ------------
        w3p = ctx.enter_context(tc.tile_pool(name="w3p", bufs=1))
        wo_sb = w3p.tile([P, CT, D_MODEL], fp8, name="wo2")
        gam_sb = w3p.tile([P, D_MODEL], f32, name="gamr")
        bet_sb = w3p.tile([P, D_MODEL], f32, name="betr")
        hq_sb = [w3p.tile([P, D_MODEL], f32, name=f"hqr{t}") for t in range(TQ)]

        # ---- phase 1 scope: h^T residency + streamed W columns --------------
        ph1_ctx = ExitStack()
        ph1 = ph1_ctx.enter_context(tc.tile_pool(name="ph1", bufs=1))
        ht_sb = ph1.tile([P, CT, SEQ], fp8, name="ht2")

        wcol = ph1_ctx.enter_context(tc.tile_pool(name="wcol", bufs=3))

        def load_wcol(w, e, tag):
            wc = wcol.tile([P, CT, P], fp8, tag=tag, name=f"{tag}{e}")
            nc.sync.dma_start(out=wc, in_=w[:, e])
            return wc

        # startup DMA priority: the first K-proj matmul needs wkc(0) + ht,
        # so those go first; ht split over the 3 DMA-capable queues.
        wc0 = load_wcol(wk2, 0, "wkc")
        nc.sync.dma_start(out=ht_sb[:, 0:2, :], in_=ht2[:, 0:2, :])
        nc.scalar.dma_start(out=ht_sb[:, 2:4, :], in_=ht2[:, 2:4, :])
        nc.gpsimd.dma_start(out=ht_sb[:, 4:6, :], in_=ht2[:, 4:6, :])
        nc.gpsimd.dma_start(out=ht_sb[:, 6:8, :], in_=ht2[:, 6:8, :])

        def kq_group(ps_ap, wc, sl):
            """4 DoubleRow matmuls: one K/Q-proj output group into psum."""
            for c in range(0, CT, 2):
                nc.tensor.matmul(ps_ap, wc[:, c:c + 2, :],
                                 ht_sb[:, c:c + 2, sl * NSL:(sl + 1) * NSL],
                                 start=(c == 0), stop=(c == CT - 2),
                                 perf_mode=DR)

        # prephase: K(0), Q(0), V (own pools, closed before attention)
        with tc.tile_pool(name="wvp", bufs=1) as wvp, \
             tc.tile_pool(name="psA", bufs=6, space="PSUM") as psA:
            wv_sb = wvp.tile([P, CT, D_MODEL], fp8, name="wv2")
            nc.sync.dma_start(out=wv_sb[:, 0:4, :], in_=wv2[:, 0:4, :])
            nc.scalar.dma_start(out=wv_sb[:, 4:8, :], in_=wv2[:, 4:8, :])
            wc = wc0
            kt_cur = ktq.tile([P, SEQ], bf16, tag="kt", name="kt0")
            qt_cur = ktq.tile([P, QLEN], bf16, tag="qt", name="qt0")
            for j in range(JS):
                ps = psA.tile([P, NSL], f32, tag="psa", name=f"psk0_{j}")
                kq_group(ps, wc, j)
                nc.vector.tensor_copy(kt_cur[:, j * NSL:(j + 1) * NSL], ps)
            wc = load_wcol(wq2, 0, "wqc")
            for i in range(IS):
                ps = psA.tile([P, NSL], f32, tag="psa", name=f"psq0_{i}")
                kq_group(ps, wc, i)
                nc.vector.tensor_copy(qt_cur[:, i * NSL:(i + 1) * NSL], ps)
            # V projection: stationary h^T slot pairs, moving Wv slabs.
            # The psum->sbuf copy scales V rows by the per-key mask z (so
            # masked keys contribute nothing), and the z/16 column (col 64
            # per head) makes the PV matmul accumulate den/16 in psum row 64.
            for t in range(JT):
                jp, sl8 = divmod(t, 2)
                for es in range(2):
                    ps = psA.tile([P, NSL], f32, tag="psa", name=f"psv{t}_{es}")
                    for c in range(0, CT, 2):
                        nc.tensor.matmul(
                            ps, ht_sb[:, c:c + 2, t * P:(t + 1) * P],
                            wv_sb[:, c:c + 2, es * NSL:(es + 1) * NSL],
                            start=(c == 0), stop=(c == CT - 2), perf_mode=DR)
                    nc.vector.tensor_scalar(
                        out=v_sb[jp][:, sl8, es * 8:(es + 1) * 8, 0:D_HEAD],
                        in0=ps[:, :].rearrange("p (h d) -> p h d", d=D_HEAD),
                        scalar1=z_sb[:, t:t + 1], scalar2=None,
                        op0=ALU.mult)
                nc.vector.tensor_scalar(
                    out=v_sb[jp][:, sl8, :, D_HEAD:DH1], in0=ones16,
                    scalar1=z_sb[:, t:t + 1], scalar2=None, op0=ALU.mult)

        def pv_mm(av, hp, jp, pts, hb, i):
            first, last = (jp == 0), (jp == JP - 1)
            h = hp * 2 + hb
            nc.tensor.matmul(
                av[hb][i][0:DH1, :], v_sb[jp][:, :, h, 0:DH1],
                pts[hb][:, :, i * NSL:(i + 1) * NSL],
                start=first, stop=last, perf_mode=DR,
                tile_position=(0, 0),
                skip_group_check=(hb + i > 0))

        def emit_pv(nc, v_sb, av, hp, jp, pts):
            for hb in range(2):
                for i in range(IS):
                    pv_mm(av, hp, jp, pts, hb, i)

        nc.scalar.dma_start(out=wo_sb, in_=wo2[:, :, :])
        nc.gpsimd.dma_start(out=gam_sb,
                            in_=bass.AP(tensor=gam, offset=0, ap=[[0, P], [1, D_MODEL]]))
        nc.gpsimd.dma_start(out=bet_sb,
                            in_=bass.AP(tensor=bet, offset=0, ap=[[0, P], [1, D_MODEL]]))
        hq_engs = [nc.sync, nc.scalar, nc.gpsimd]
        for t in range(TQ):
            hq_engs[t % 3].dma_start(out=hq_sb[t], in_=hq[t * P:(t + 1) * P, :])

        # ---- attention ------------------------------------------------------
        attn_ctx = ExitStack()
        scp = attn_ctx.enter_context(tc.tile_pool(name="scp", bufs=2, space="PSUM"))
        avp = attn_ctx.enter_context(tc.tile_pool(name="avp", bufs=4, space="PSUM"))
        ptp = attn_ctx.enter_context(tc.tile_pool(name="ptp", bufs=6))
        # two pairs of avc/rep are in flight at once (stage 2 of pair hp
        # runs at pair hp+1's end) — slot counts must cover both.
        nrmA = attn_ctx.enter_context(tc.tile_pool(name="nrmA", bufs=8))
        nrmB = attn_ctx.enter_context(tc.tile_pool(name="nrmB", bufs=6))
        nrmC = attn_ctx.enter_context(tc.tile_pool(name="nrmC", bufs=3))
        nrmT = attn_ctx.enter_context(tc.tile_pool(name="nrmT", bufs=2))

        # Normalize: two-stage software pipeline (see module docstring).
        def norm_stage1(hp, av):
            avcs, reps = [], []
            for i in range(IS):
                for hb in range(2):
                    avc = nrmA.tile([P, NSL], f32, tag="avc",
                                   name=f"avc{hp}_{hb}_{i}")
                    nc.vector.tensor_copy(avc[0:DH1, :], av[hb][i][0:DH1, :])
                    avcs.append(avc)
            for idx, avc in enumerate(avcs):
                rep = nrmB.tile([P, NSL], f32, tag="rep", name=f"rep{hp}_{idx}")
                # 32-lane-parallel reciprocal of den/16 -> 16/den (~1.4us)
                tmp = nrmT.tile([P, NSL], f32, tag="tmp", name=f"tm{hp}_{idx}")
                nc.vector.transpose(tmp[0:32, :], avc[64:96, :])
                nc.vector.reciprocal(
                    tmp[0:32, :].rearrange("p (b c) -> p c b", c=32)[:, 0:1, :],
                    tmp[0:32, :].rearrange("p (b c) -> p c b", c=32)[:, 0:1, :])
                nc.vector.transpose(rep[0:32, :], tmp[0:32, :])
                reps.append(rep)
            return avcs, reps

        def norm_stage2(hp, avcs, reps):
            ee, sl2 = divmod(hp, 2)
            for i in range(IS):
                for hb in range(2):
                    idx = i * 2 + hb
                    sl = slice(i * NSL, (i + 1) * NSL)
                    repl = nrmC.tile([P, NSL], f32, tag="repl",
                                    name=f"repl{hp}_{hb}_{i}")
                    nc.gpsimd.partition_broadcast(repl[0:64, :],
                                                  reps[idx][0:1, :])
                    if hb == 0:
                        nc.vector.tensor_mul(avt_sb[ee][0:64, sl2, sl],
                                             avcs[idx][0:64, :],
                                             repl[0:64, :])
                    else:
                        navt = nrmC.tile([P, NSL], fp8, tag="navt",
                                        name=f"navt{hp}_{i}")
                        nc.vector.tensor_mul(navt[0:64, :],
                                             avcs[idx][0:64, :],
                                             repl[0:64, :])
                        eng = nc.sync if i == 0 else nc.scalar
                        eng.dma_start(out=avt_sb[ee][64:P, sl2, sl],
                                      in_=navt[0:64, :])

        pending = None

        for hp in range(HP):
            av = [[avp.tile([P, NSL], f32, tag="av", name=f"av{hp}_{hb}_{i}")
                   for i in range(IS)] for hb in range(2)]
            # interleaved projection work for the NEXT head pair, one
            # 4-matmul group per even j (covered by the two buffered exps),
            # borrowing a scores-pool psum slot: j -> (kind, slab)
            proj_work = ({4: ("k", 0), 6: ("k", 1), 8: ("k", 2), 10: ("k", 3),
                          12: ("q", 0), 14: ("q", 1)} if hp + 1 < HP else {})
            # PV (jtile-pair jp) runs ~2 j behind the scores/exp front,
            # split 2+2 around the exps so the 4-matmul burst never starves
            # the ACT queue.
            pt_q = {}
            wc_k = wc_q = None
            kt_nxt = qt_nxt = None

            for j in range(JT):
                jp, sl2 = divmod(j, 2)
                if sl2 == 0:
                    pt_q[jp] = [ptp.tile([P, 2, QLEN], fp8, tag="pt",
                                         name=f"pt{hp}_{jp}_{hb}")
                                for hb in range(2)]
                pv_jp = (j - 3) // 2 if (j >= 3 and j % 2 == 1) else None
                # hb-major issue: exp(hb) queues right after its two score
                # matmuls, keeping the scores->exp->next-scores chain short.
                for hb in range(2):
                    base = hb * 64
                    sc = scp.tile([P, QLEN], f32, tag="sc",
                                  name=f"sc{hp}_{j}_{hb}")
                    for i in range(IS):
                        nc.tensor.matmul(
                            sc[:, i * NSL:(i + 1) * NSL],
                            kt_cur[base:base + 64, j * P:(j + 1) * P],
                            qt_cur[base:base + 64, i * NSL:(i + 1) * NSL],
                            start=True, stop=True, tile_position=(base, 0))
                    nc.scalar.activation(pt_q[jp][hb][:, sl2, :], sc,
                                         AF.Exp, scale=SCALE,
                                         bias=expb_sb[:, 0:1])
                    if pv_jp is not None:
                        for i in range(IS):
                            pv_mm(av, hp, pv_jp, pt_q[pv_jp], hb, i)

                if j == 3 and proj_work:
                    wc_k = load_wcol(wk2, hp + 1, "wkc")
                    kt_nxt = ktq.tile([P, SEQ], bf16, tag="kt",
                                      name=f"kt{hp + 1}")
                if j == 11 and proj_work:
                    wc_q = load_wcol(wq2, hp + 1, "wqc")
                    qt_nxt = ktq.tile([P, QLEN], bf16, tag="qt",
                                      name=f"qt{hp + 1}")
                if j in proj_work:
                    kind, sl = proj_work[j]
                    borrow = scp.tile([P, QLEN], f32, tag="sc",
                                      name=f"bw{hp}_{j}")
                    dst = kt_nxt if kind == "k" else qt_nxt
                    kq_group(borrow[:, 0:NSL], wc_k if kind == "k" else wc_q, sl)
                    nc.vector.tensor_copy(
                        dst[:, sl * NSL:(sl + 1) * NSL], borrow[:, 0:NSL])

            # drain the PV pipeline (jp JP-1; jp 0..JP-2 emitted in-loop)
            emit_pv(nc, v_sb, av, hp, JP - 1, pt_q[JP - 1])

            # evacuate THIS pair's av psum first, then complete the
            # PREVIOUS pair's deferred normalize.
            cur = norm_stage1(hp, av)
            if pending is not None:
                norm_stage2(hp - 1, *pending)
            pending = cur
            if kt_nxt is not None:
                kt_cur, qt_cur = kt_nxt, qt_nxt

        norm_stage2(HP - 1, *pending)

        # ---- output projection + residual + layernorm -----------------------
        attn_ctx.close()
        ph1_ctx.close()

        pso = ctx.enter_context(tc.tile_pool(name="pso", bufs=8, space="PSUM"))
        lnp = ctx.enter_context(tc.tile_pool(name="lnp", bufs=3))
        lns = ctx.enter_context(tc.tile_pool(name="lns", bufs=8))

        for t in range(TQ):
            hq_t = hq_sb[t]
            xs = lnp.tile([P, D_MODEL], f32, tag="xs", name=f"xs{t}")
            sums = lns.tile([P, 2], f32, tag="sm", name=f"sm{t}")
            for m in range(2):
                ps = pso.tile([P, NSL], f32, tag="po", name=f"po{t}_{m}")
                for ee in range(ET // 2):
                    nc.tensor.matmul(
                        ps, avt_sb[ee][:, :, t * P:(t + 1) * P],
                        wo_sb[:, 2 * ee:2 * ee + 2, m * NSL:(m + 1) * NSL],
                        start=(ee == 0), stop=(ee == ET // 2 - 1),
                        perf_mode=DR)
                nc.vector.scalar_tensor_tensor(
                    out=xs[:, m * NSL:(m + 1) * NSL], in0=ps, scalar=1.0 / AVS,
                    in1=hq_t[:, m * NSL:(m + 1) * NSL],
                    op0=ALU.mult, op1=ALU.add,
                    accum_out=sums[:, m:m + 1])
            # mean/var via accum sums + ACT Square pass:
            # mean = (s0+s1)/D; var = sq/D - mean^2
            sq = lns.tile([P, 2], f32, tag="sq", name=f"sq{t}")
            xsq = lnp.tile([P, D_MODEL], f32, tag="xq", name=f"xq{t}")
            for m in range(2):
                nc.scalar.activation(xsq[:, m * NSL:(m + 1) * NSL],
                                     xs[:, m * NSL:(m + 1) * NSL], AF.Square,
                                     accum_out=sq[:, m:m + 1])
            mean = lns.tile([P, 1], f32, tag="mn", name=f"mn{t}")
            nc.vector.tensor_add(mean, sums[:, 0:1], sums[:, 1:2])
            nc.vector.tensor_scalar_mul(mean, mean, 1.0 / D_MODEL)
            msq = lns.tile([P, 1], f32, tag="mq", name=f"mq{t}")
            nc.vector.tensor_mul(msq, mean, mean)
            var = lns.tile([P, 1], f32, tag="vr", name=f"vr{t}")
            nc.vector.tensor_add(var, sq[:, 0:1], sq[:, 1:2])
            nc.vector.scalar_tensor_tensor(
                out=var, in0=var, scalar=1.0 / D_MODEL, in1=msq,
                op0=ALU.mult, op1=ALU.subtract)
            std = lns.tile([P, 1], f32, tag="sd", name=f"sd{t}")
            nc.scalar.activation(std, var, AF.Sqrt, bias=eps_sb[:, 0:1])
            rstd = lns.tile([P, 1], f32, tag="rs", name=f"rs{t}")
            nc.vector.reciprocal(rstd, std)
            nmr = lns.tile([P, 1], f32, tag="nm", name=f"nm{t}")
            nc.vector.tensor_scalar_mul(nmr, mean, -1.0)
            # gs on ACT (Copy with per-partition scale), xn on gpsimd
            gs = lnp.tile([P, D_MODEL], f32, tag="gs", name=f"gs{t}")
            nc.scalar.activation(gs, gam_sb, AF.Copy, scale=rstd[:, 0:1])
            xg = lnp.tile([P, D_MODEL], f32, tag="xg", name=f"xg{t}")
            nc.vector.scalar_tensor_tensor(
                out=xg, in0=xs, scalar=nmr[:, 0:1], in1=gs,
                op0=ALU.add, op1=ALU.mult)
            xn = lnp.tile([P, D_MODEL], f32, tag="xn", name=f"xn{t}")
            nc.gpsimd.tensor_add(xn, xg, bet_sb)
            nc.sync.dma_start(out=out[t * P:(t + 1) * P, :], in_=xn)

    nc.compile()
    return nc


def _get_nc():
    if "nc" not in _CACHE:
        _CACHE["nc"] = _build()
    return _CACHE["nc"]


def _make_in_maps(inputs):
    fp8 = ml_dtypes.float8_e4m3
    h = np.asarray(inputs["h"], dtype=np.float32)
    mask = np.asarray(inputs["attn_mask"])
    Wq = np.asarray(inputs["Wq"], dtype=np.float32)
    Wkv = np.asarray(inputs["Wkv"], dtype=np.float32)
    Wo = np.asarray(inputs["Wo"], dtype=np.float32)
    gamma = np.asarray(inputs["gamma"], dtype=np.float32)
    beta = np.asarray(inputs["beta"], dtype=np.float32)

    def wpack(W):
        # [D, D] -> [128, CT, D]: wpack[p, c, e] = W[c*128+p, e]
        return np.ascontiguousarray(
            W.reshape(CT, P, D_MODEL).transpose(1, 0, 2).astype(fp8))

    def wpack_e(W):
        # [D, D] -> [128, ET, CT, 128]: [p, e, c, ce] = W[c*128+p, e*128+ce]
        return np.ascontiguousarray(
            W.reshape(CT, P, ET, P).transpose(1, 2, 0, 3).astype(fp8))

    wq_p = wpack_e(Wq)
    wk_p = wpack_e(Wkv[:, :D_MODEL])
    wv_p = wpack(Wkv[:, D_MODEL:])
    wo_p = wpack(Wo)

    in_maps = []
    for c in range(8):
        b, half = divmod(c, 2)
        hb = h[:, b, :]
        own = slice(half * QLEN, (half + 1) * QLEN)
        other = slice((1 - half) * QLEN, (2 - half) * QLEN)
        # own query-half first: keys are in core-local order, so the Q
        # projection can read the first half of hT uniformly on every core.
        hT_r = np.concatenate([hb[own, :], hb[other, :]], axis=0).T
        # [D, S] -> [128, CT, S]
        ht_p = np.ascontiguousarray(
            hT_r.reshape(CT, P, SEQ).transpose(1, 0, 2).astype(fp8))
        z_full = np.where(mask[:, b], np.float32(0.0), np.float32(1.0))
        in_maps.append({
            "ht2": ht_p,
            "hq": np.ascontiguousarray(hb[own, :]),
            "wq2": wq_p, "wk2": wk_p, "wv2": wv_p, "wo2": wo_p,
            "zt": np.ascontiguousarray(
                np.concatenate([z_full[own], z_full[other]])),
            "gam": gamma, "bet": beta,
        })
    return in_maps


def _run(in_maps, **kwargs):
    from concourse.bass_utils import run_bass_kernel_spmd
    return run_bass_kernel_spmd(_get_nc(), in_maps, core_ids=list(range(8)),
                                **kwargs)


def kernel(**inputs) -> np.ndarray:
    res = _run(_make_in_maps(inputs))
    out = np.empty((SEQ, BSZ, D_MODEL), dtype=np.float32)
    for c in range(8):
        b, half = divmod(c, 2)
        out[half * QLEN:(half + 1) * QLEN, :, :][:, b, :] = res.results[c]["out"]
    return out


# revision 24
# speedup vs baseline: 1.0059x; 1.0059x over previous
"""MultiHeadAttn (post-LN, key-padding mask) Trainium2 Bass kernel, 8 cores.

Problem: h [S=2048, B=4, D=1024] f32; 16 heads x 64; key-padding mask [S, B];
out = LayerNorm(h + MHA(h)).

Sharding: core c handles batch b = c//2 and query half qh = c%2 (1024 query
rows), with all 16 heads and the full 2048-key context for that batch.
KV projections are recomputed by both cores of a batch pair (no collectives).

Per-core device pipeline:
  - All four projections (K^T, Q^T, V, O) and the PV matmul run in fp8e4
    with perf_mode=DoubleRow: operands are [128, 2, free] slot-pair APs so
    each 512-col moving stream contracts 256 rows -> half the PE streams of
    the bf16 version.  Scores stay bf16 (K^T/Q^T psum evacuated to bf16).
  - V proj psum is evacuated by the (otherwise idle in the prephase) ACT
    engine via Copy with per-partition scale z in {0,1} (masked key -> 0);
    a z*(1/16) column is appended per head (col 64), so the DoubleRow PV
    matmul (stationary [z*V | z/16], M=65) accumulates the numerator and,
    in psum row 64, den/16.  The 1/16 keeps the fp8 avt in the normal
    e4m3 range (avt stores 16*attn_vec; the output projection's residual
    STT multiplies by 1/16).
  - Attention per head pair: scores^T [j,i] via row-paired bf16 matmuls
    (two heads in row strips 0-63 / 64-127, i-major issue order), exp via
    ScalarE with scale=1/8 and bias=-2 (softmax-invariant shift keeping
    exp values < 105 so the fp8e4 pt never overflows), written straight to
    fp8 slot j%2 of a [128, 2, 1024] pt tile.  PV consumes jtile PAIRS
    (DoubleRow contraction 256 keys) ~2 j behind the scores/exp front.
  - Normalize (two-stage software pipeline, all off the PE critical path):
    stage 1 evacuates av psum via DVE copies and computes 16/den with a
    32-lane-parallel transpose/strided-reciprocal/transpose; stage 2 -
    deferred to the NEXT pair's end - broadcasts 16/den to 64 rows on
    GPSIMD and multiplies into the fp8 avt slot (head1 via a small
    partition-shift DMA into rows 64-127).
  - Output proj: DoubleRow over e-tile pairs; residual add + LN spread
    over DVE (stt/accum), ScalarE (Square, Sqrt, gamma*rstd) and GPSIMD
    (+beta); 8 psum banks + 3-deep tile pipeline hide the chain.
Next head pair's K/Q projections are interleaved into the attention loop
(borrowing scores-pool PSUM slots) so the PE stays busy under the ACT-bound
softmax stream.
"""
import numpy as np
import ml_dtypes

N_HEAD, D_MODEL, D_HEAD = 16, 1024, 64
SEQ, BSZ = 2048, 4
QLEN = SEQ // 2
SCALE = 1.0 / D_HEAD ** 0.5
EXP_BIAS = -2.0             # softmax-invariant shift; max score 6.65 -> exp<105
LN_EPS = 1e-5
P = 128
NSL = 512                   # matmul moving-operand slab (one PSUM bank fp32)
CT = D_MODEL // P           # 8 contraction tiles
ET = D_MODEL // P           # 8 e-tiles (2 heads each)
JT = SEQ // P               # 16 key tiles
JP = JT // 2                # 8 key-tile pairs (DoubleRow PV)
JS = SEQ // NSL             # 4 key slabs
IS = QLEN // NSL            # 2 query slabs
TQ = QLEN // P              # 8 query-row tiles
HP = N_HEAD // 2            # 8 head pairs
DH1 = D_HEAD + 1            # V columns per head incl. the z (denominator) col
DHP = 80                    # padded per-head V stride (16B-aligned fp8 slots)
AVS = 16.0                  # avt stores AVS*attn_vec (fp8 range use)

_CACHE = {}


def _build():
    from contextlib import ExitStack
    import concourse.bass as bass
    import concourse.mybir as mybir
    import concourse.tile as tile
    from concourse import bacc

    dt = mybir.dt
    f32, bf16, fp8 = dt.float32, dt.bfloat16, dt.float8e4
    AF = mybir.ActivationFunctionType
    ALU = mybir.AluOpType
    DR = mybir.MatmulPerfMode.DoubleRow

    nc = bacc.Bacc(None, target_bir_lowering=False)

    # fp8 host layouts: ht2[p, c, s] = h^T[c*128+p, s]; wq2/wk2[p, e, c, ce] =
    # W[c*128+p, e*128+ce] (e-tile-major for contiguous column loads);
    # wv2/wo2[p, c, e] = W[c*128+p, e].
    ht2 = nc.dram_tensor("ht2", [P, CT, SEQ], fp8, kind="ExternalInput")
    hq = nc.dram_tensor("hq", [QLEN, D_MODEL], f32, kind="ExternalInput")
    wq2 = nc.dram_tensor("wq2", [P, ET, CT, P], fp8, kind="ExternalInput")
    wk2 = nc.dram_tensor("wk2", [P, ET, CT, P], fp8, kind="ExternalInput")
    wv2 = nc.dram_tensor("wv2", [P, CT, D_MODEL], fp8, kind="ExternalInput")
    wo2 = nc.dram_tensor("wo2", [P, CT, D_MODEL], fp8, kind="ExternalInput")
    zt = nc.dram_tensor("zt", [SEQ], f32, kind="ExternalInput")
    gam = nc.dram_tensor("gam", [D_MODEL], f32, kind="ExternalInput")
    bet = nc.dram_tensor("bet", [D_MODEL], f32, kind="ExternalInput")
    out = nc.dram_tensor("out", [QLEN, D_MODEL], f32, kind="ExternalOutput")

    with tile.TileContext(nc) as tc, ExitStack() as ctx:
        persist = ctx.enter_context(tc.tile_pool(name="persist", bufs=1))

        # K/Q tiles die after their pair's scores — roll through 2 slots
        ktq = ctx.enter_context(tc.tile_pool(name="ktq", bufs=2))
        # V in fp8 slot pairs: v2[p, slot, head, 0:64]=z*V, [.., 64]=z/16
        v_sb = [persist.tile([P, 2, N_HEAD, DHP], fp8, name=f"v{t}")
                for t in range(JP)]
        # avt in fp8 slot pairs for the DoubleRow output projection
        avt_sb = [persist.tile([P, 2, QLEN], fp8, name=f"avt{e}")
                  for e in range(ET // 2)]
        z_sb = persist.tile([P, JT], f32, name="zmask")
        ones16 = persist.tile([P, N_HEAD, 1], f32, name="ones16")
        eps_sb = persist.tile([P, 1], f32, name="eps")
        expb_sb = persist.tile([P, 1], f32, name="expb")

        nc.vector.memset(eps_sb, LN_EPS)
        nc.vector.memset(expb_sb, EXP_BIAS)
        nc.vector.memset(ones16, 1.0 / AVS)

        nc.gpsimd.dma_start(out=z_sb,
                            in_=bass.AP(tensor=zt, offset=0, ap=[[1, P], [P, JT]]))

        # ---- phase-3 weights + residual input (prefetched early) ------------
        w3p = ctx.enter_context(tc.tile_pool(name="w3p", bufs=1))
        wo_sb = w3p.tile([P, CT, D_MODEL], fp8, name="wo2")
        gam_sb = w3p.tile([P, D_MODEL], f32, name="gamr")
        bet_sb = w3p.tile([P, D_MODEL], f32, name="betr")
        hq_sb = [w3p.tile([P, D_MODEL], f32, name=f"hqr{t}") for t in range(TQ)]

        # ---- phase 1 scope: h^T residency + streamed W columns --------------
        ph1_ctx = ExitStack()
        ph1 = ph1_ctx.enter_context(tc.tile_pool(name="ph1", bufs=1))
        ht_sb = ph1.tile([P, CT, SEQ], fp8, name="ht2")

        wcol = ph1_ctx.enter_context(tc.tile_pool(name="wcol", bufs=3))

        def load_wcol(w, e, tag):
            wc = wcol.tile([P, CT, P], fp8, tag=tag, name=f"{tag}{e}")
            nc.sync.dma_start(out=wc, in_=w[:, e])
            return wc

        # startup DMA priority: the first K-proj matmul needs wkc(0) + ht,
        # so those go first; ht split over the 3 DMA-capable queues.
        wc0 = load_wcol(wk2, 0, "wkc")
        nc.sync.dma_start(out=ht_sb[:, 0:2, :], in_=ht2[:, 0:2, :])
        nc.scalar.dma_start(out=ht_sb[:, 2:4, :], in_=ht2[:, 2:4, :])
        nc.gpsimd.dma_start(out=ht_sb[:, 4:6, :], in_=ht2[:, 4:6, :])
        nc.gpsimd.dma_start(out=ht_sb[:, 6:8, :], in_=ht2[:, 6:8, :])

        def kq_group(ps_ap, wc, sl):
            """4 DoubleRow matmuls: one K/Q-proj output group into psum."""
            for c in range(0, CT, 2):
                nc.tensor.matmul(ps_ap, wc[:, c:c + 2, :],
                                 ht_sb[:, c:c + 2, sl * NSL:(sl + 1) * NSL],
                                 start=(c == 0), stop=(c == CT - 2),
                                 perf_mode=DR)

        # prephase: K(0), Q(0), V (own pools, closed before attention)
        with tc.tile_pool(name="wvp", bufs=1) as wvp, \
             tc.tile_pool(name="psA", bufs=6, space="PSUM") as psA:
            wv_sb = wvp.tile([P, CT, D_MODEL], fp8, name="wv2")
            nc.sync.dma_start(out=wv_sb[:, 0:4, :], in_=wv2[:, 0:4, :])
            nc.scalar.dma_start(out=wv_sb[:, 4:8, :], in_=wv2[:, 4:8, :])
            wc = wc0
            kt_cur = ktq.tile([P, SEQ], bf16, tag="kt", name="kt0")
            qt_cur = ktq.tile([P, QLEN], bf16, tag="qt", name="qt0")
            for j in range(JS):
                ps = psA.tile([P, NSL], f32, tag="psa", name=f"psk0_{j}")
                kq_group(ps, wc, j)
                nc.vector.tensor_copy(kt_cur[:, j * NSL:(j + 1) * NSL], ps)
            wc = load_wcol(wq2, 0, "wqc")
            for i in range(IS):
                ps = psA.tile([P, NSL], f32, tag="psa", name=f"psq0_{i}")
                kq_group(ps, wc, i)
                nc.vector.tensor_copy(qt_cur[:, i * NSL:(i + 1) * NSL], ps)
            # V projection: stationary h^T slot pairs, moving Wv slabs.
            # The psum->sbuf copy scales V rows by the per-key mask z (so
            # masked keys contribute nothing), and the z/16 column (col 64
            # per head) makes the PV matmul accumulate den/16 in psum row 64.
            for t in range(JT):
                jp, sl8 = divmod(t, 2)
                for es in range(2):
                    ps = psA.tile([P, NSL], f32, tag="psa", name=f"psv{t}_{es}")
                    for c in range(0, CT, 2):
                        nc.tensor.matmul(
                            ps, ht_sb[:, c:c + 2, t * P:(t + 1) * P],
                            wv_sb[:, c:c + 2, es * NSL:(es + 1) * NSL],
                            start=(c == 0), stop=(c == CT - 2), perf_mode=DR)
                    nc.vector.tensor_scalar(
                        out=v_sb[jp][:, sl8, es * 8:(es + 1) * 8, 0:D_HEAD],
                        in0=ps[:, :].rearrange("p (h d) -> p h d", d=D_HEAD),
                        scalar1=z_sb[:, t:t + 1], scalar2=None,
                        op0=ALU.mult)
                nc.vector.tensor_scalar(
                    out=v_sb[jp][:, sl8, :, D_HEAD:DH1], in0=ones16,
                    scalar1=z_sb[:, t:t + 1], scalar2=None, op0=ALU.mult)

        def pv_mm(av, hp, jp, pts, hb, i):
            first, last = (jp == 0), (jp == JP - 1)
            h = hp * 2 + hb
            nc.tensor.matmul(
                av[hb][i][0:DH1, :], v_sb[jp][:, :, h, 0:DH1],
                pts[hb][:, :, i * NSL:(i + 1) * NSL],
                start=first, stop=last, perf_mode=DR,
                tile_position=(0, 0),
                skip_group_check=(hb + i > 0))

        def emit_pv(nc, v_sb, av, hp, jp, pts):
            for hb in range(2):
                for i in range(IS):
                    pv_mm(av, hp, jp, pts, hb, i)

        nc.scalar.dma_start(out=wo_sb, in_=wo2[:, :, :])
        nc.gpsimd.dma_start(out=gam_sb,
                            in_=bass.AP(tensor=gam, offset=0, ap=[[0, P], [1, D_MODEL]]))
        nc.gpsimd.dma_start(out=bet_sb,
                            in_=bass.AP(tensor=bet, offset=0, ap=[[0, P], [1, D_MODEL]]))
        hq_engs = [nc.sync, nc.scalar, nc.gpsimd]
        for t in range(TQ):
            hq_engs[t % 3].dma_start(out=hq_sb[t], in_=hq[t * P:(t + 1) * P, :])

        # ---- attention ------------------------------------------------------
        attn_ctx = ExitStack()
        scp = attn_ctx.enter_context(tc.tile_pool(name="scp", bufs=2, space="PSUM"))
        avp = attn_ctx.enter_context(tc.tile_pool(name="avp", bufs=4, space="PSUM"))
        ptp = attn_ctx.enter_context(tc.tile_pool(name="ptp", bufs=8))
        # two pairs of avc/rep are in flight at once (stage 2 of pair hp
        # runs at pair hp+1's end) — slot counts must cover both.
        nrmA = attn_ctx.enter_context(tc.tile_pool(name="nrmA", bufs=8))
        nrmB = attn_ctx.enter_context(tc.tile_pool(name="nrmB", bufs=6))
        nrmC = attn_ctx.enter_context(tc.tile_pool(name="nrmC", bufs=3))
        nrmT = attn_ctx.enter_context(tc.tile_pool(name="nrmT", bufs=2))

        # Normalize: two-stage software pipeline (see module docstring).
        def norm_stage1(hp, av):
            avcs, reps = [], []
            for i in range(IS):
                for hb in range(2):
                    avc = nrmA.tile([P, NSL], f32, tag="avc",
                                   name=f"avc{hp}_{hb}_{i}")
                    nc.vector.tensor_copy(avc[0:DH1, :], av[hb][i][0:DH1, :])
                    avcs.append(avc)
            for idx, avc in enumerate(avcs):
                rep = nrmB.tile([P, NSL], f32, tag="rep", name=f"rep{hp}_{idx}")
                # 32-lane-parallel reciprocal of den/16 -> 16/den (~1.4us)
                tmp = nrmT.tile([P, NSL], f32, tag="tmp", name=f"tm{hp}_{idx}")
                nc.vector.transpose(tmp[0:32, :], avc[64:96, :])
                nc.vector.reciprocal(
                    tmp[0:32, :].rearrange("p (b c) -> p c b", c=32)[:, 0:1, :],
                    tmp[0:32, :].rearrange("p (b c) -> p c b", c=32)[:, 0:1, :])
                nc.vector.transpose(rep[0:32, :], tmp[0:32, :])
                reps.append(rep)
            return avcs, reps

        def norm_stage2(hp, avcs, reps):
            ee, sl2 = divmod(hp, 2)
            for i in range(IS):
                for hb in range(2):
                    idx = i * 2 + hb
                    sl = slice(i * NSL, (i + 1) * NSL)
                    repl = nrmC.tile([P, NSL], f32, tag="repl",
                                    name=f"repl{hp}_{hb}_{i}")
                    nc.gpsimd.partition_broadcast(repl[0:64, :],
                                                  reps[idx][0:1, :])
                    if hb == 0:
                        nc.vector.tensor_mul(avt_sb[ee][0:64, sl2, sl],
                                             avcs[idx][0:64, :],
                                             repl[0:64, :])
                    else:
                        navt = nrmC.tile([P, NSL], fp8, tag="navt",
                                        name=f"navt{hp}_{i}")
                        nc.vector.tensor_mul(navt[0:64, :],
                                             avcs[idx][0:64, :],
                                             repl[0:64, :])
                        eng = nc.sync if i == 0 else nc.scalar
                        eng.dma_start(out=avt_sb[ee][64:P, sl2, sl],
                                      in_=navt[0:64, :])

        pending = None

        for hp in range(HP):
            av = [[avp.tile([P, NSL], f32, tag="av", name=f"av{hp}_{hb}_{i}")
                   for i in range(IS)] for hb in range(2)]
            # interleaved projection work for the NEXT head pair, one
            # 4-matmul group per even j (covered by the two buffered exps),
            # borrowing a scores-pool psum slot: j -> (kind, slab)
            proj_work = ({4: ("k", 0), 6: ("k", 1), 8: ("k", 2), 10: ("k", 3),
                          12: ("q", 0), 14: ("q", 1)} if hp + 1 < HP else {})
            # PV (jtile-pair jp) runs ~2 j behind the scores/exp front,
            # split 2+2 around the exps so the 4-matmul burst never starves
            # the ACT queue.
            pt_q = {}
            wc_k = wc_q = None
            kt_nxt = qt_nxt = None

            for j in range(JT):
                jp, sl2 = divmod(j, 2)
                if sl2 == 0:
                    pt_q[jp] = [ptp.tile([P, 2, QLEN], fp8, tag="pt",
                                         name=f"pt{hp}_{jp}_{hb}")
                                for hb in range(2)]
                # hb-major issue: exp(hb) queues right after its two score
                # matmuls, keeping the scores->exp->next-scores chain short.
                for hb in range(2):
                    base = hb * 64
                    sc = scp.tile([P, QLEN], f32, tag="sc",
                                  name=f"sc{hp}_{j}_{hb}")
                    for i in range(IS):
                        nc.tensor.matmul(
                            sc[:, i * NSL:(i + 1) * NSL],
                            kt_cur[base:base + 64, j * P:(j + 1) * P],
                            qt_cur[base:base + 64, i * NSL:(i + 1) * NSL],
                            start=True, stop=True, tile_position=(base, 0))
                    nc.scalar.activation(pt_q[jp][hb][:, sl2, :], sc,
                                         AF.Exp, scale=SCALE,
                                         bias=expb_sb[:, 0:1])
                if j >= 5 and j % 2 == 1:
                    emit_pv(nc, v_sb, av, hp, (j - 5) // 2, pt_q[(j - 5) // 2])

                if j == 3 and proj_work:
                    wc_k = load_wcol(wk2, hp + 1, "wkc")
                    kt_nxt = ktq.tile([P, SEQ], bf16, tag="kt",
                                      name=f"kt{hp + 1}")
                if j == 11 and proj_work:
                    wc_q = load_wcol(wq2, hp + 1, "wqc")
                    qt_nxt = ktq.tile([P, QLEN], bf16, tag="qt",
                                      name=f"qt{hp + 1}")
                if j in proj_work:
                    kind, sl = proj_work[j]
                    borrow = scp.tile([P, QLEN], f32, tag="sc",
                                      name=f"bw{hp}_{j}")
                    dst = kt_nxt if kind == "k" else qt_nxt
                    kq_group(borrow[:, 0:NSL], wc_k if kind == "k" else wc_q, sl)
                    nc.vector.tensor_copy(
                        dst[:, sl * NSL:(sl + 1) * NSL], borrow[:, 0:NSL])

            # drain the PV pipeline (jp 0..JP-3 emitted in-loop)
            emit_pv(nc, v_sb, av, hp, JP - 2, pt_q[JP - 2])
            emit_pv(nc, v_sb, av, hp, JP - 1, pt_q[JP - 1])

            # evacuate THIS pair's av psum first, then complete the
            # PREVIOUS pair's deferred normalize.
            cur = norm_stage1(hp, av)
            if pending is not None:
                norm_stage2(hp - 1, *pending)
            pending = cur
            if kt_nxt is not None:
                kt_cur, qt_cur = kt_nxt, qt_nxt

        norm_stage2(HP - 1, *pending)

        # ---- output projection + residual + layernorm -----------------------
        attn_ctx.close()
        ph1_ctx.close()

        pso = ctx.enter_context(tc.tile_pool(name="pso", bufs=8, space="PSUM"))
        lnp = ctx.enter_context(tc.tile_pool(name="lnp", bufs=3))
        lns = ctx.enter_context(tc.tile_pool(name="lns", bufs=8))

        for t in range(TQ):
            hq_t = hq_sb[t]
            xs = lnp.tile([P, D_MODEL], f32, tag="xs", name=f"xs{t}")
            sums = lns.tile([P, 2], f32, tag="sm", name=f"sm{t}")
            for m in range(2):
                ps = pso.tile([P, NSL], f32, tag="po", name=f"po{t}_{m}")
                for ee in range(ET // 2):
                    nc.tensor.matmul(
                        ps, avt_sb[ee][:, :, t * P:(t + 1) * P],
                        wo_sb[:, 2 * ee:2 * ee + 2, m * NSL:(m + 1) * NSL],
                        start=(ee == 0), stop=(ee == ET // 2 - 1),
                        perf_mode=DR)
                nc.vector.scalar_tensor_tensor(
                    out=xs[:, m * NSL:(m + 1) * NSL], in0=ps, scalar=1.0 / AVS,
                    in1=hq_t[:, m * NSL:(m + 1) * NSL],
                    op0=ALU.mult, op1=ALU.add,
                    accum_out=sums[:, m:m + 1])
            # mean/var via accum sums + ACT Square pass:
            # mean = (s0+s1)/D; var = sq/D - mean^2
            sq = lns.tile([P, 2], f32, tag="sq", name=f"sq{t}")
            xsq = lnp.tile([P, D_MODEL], f32, tag="xq", name=f"xq{t}")
            for m in range(2):
                nc.scalar.activation(xsq[:, m * NSL:(m + 1) * NSL],
                                     xs[:, m * NSL:(m + 1) * NSL], AF.Square,
                                     accum_out=sq[:, m:m + 1])
            mean = lns.tile([P, 1], f32, tag="mn", name=f"mn{t}")
            nc.vector.tensor_add(mean, sums[:, 0:1], sums[:, 1:2])
            nc.vector.tensor_scalar_mul(mean, mean, 1.0 / D_MODEL)
            msq = lns.tile([P, 1], f32, tag="mq", name=f"mq{t}")
            nc.vector.tensor_mul(msq, mean, mean)
            var = lns.tile([P, 1], f32, tag="vr", name=f"vr{t}")
            nc.vector.tensor_add(var, sq[:, 0:1], sq[:, 1:2])
            nc.vector.scalar_tensor_tensor(
                out=var, in0=var, scalar=1.0 / D_MODEL, in1=msq,
                op0=ALU.mult, op1=ALU.subtract)
            std = lns.tile([P, 1], f32, tag="sd", name=f"sd{t}")
            nc.scalar.activation(std, var, AF.Sqrt, bias=eps_sb[:, 0:1])
            rstd = lns.tile([P, 1], f32, tag="rs", name=f"rs{t}")
            nc.vector.reciprocal(rstd, std)
            nmr = lns.tile([P, 1], f32, tag="nm", name=f"nm{t}")
            nc.vector.tensor_scalar_mul(nmr, mean, -1.0)
            # gs on ACT (Copy with per-partition scale), xn on gpsimd
            gs = lnp.tile([P, D_MODEL], f32, tag="gs", name=f"gs{t}")
            nc.scalar.activation(gs, gam_sb, AF.Copy, scale=rstd[:, 0:1])
            xg = lnp.tile([P, D_MODEL], f32, tag="xg", name=f"xg{t}")
            nc.vector.scalar_tensor_tensor(
                out=xg, in0=xs, scalar=nmr[:, 0:1], in1=gs,
                op0=ALU.add, op1=ALU.mult)
            xn = lnp.tile([P, D_MODEL], f32, tag="xn", name=f"xn{t}")
            nc.gpsimd.tensor_add(xn, xg, bet_sb)
            nc.sync.dma_start(out=out[t * P:(t + 1) * P, :], in_=xn)

    nc.compile()
    return nc


def _get_nc():
    if "nc" not in _CACHE:
        _CACHE["nc"] = _build()
    return _CACHE["nc"]


def _make_in_maps(inputs):
    fp8 = ml_dtypes.float8_e4m3
    h = np.asarray(inputs["h"], dtype=np.float32)
    mask = np.asarray(inputs["attn_mask"])
    Wq = np.asarray(inputs["Wq"], dtype=np.float32)
    Wkv = np.asarray(inputs["Wkv"], dtype=np.float32)
    Wo = np.asarray(inputs["Wo"], dtype=np.float32)
    gamma = np.asarray(inputs["gamma"], dtype=np.float32)
    beta = np.asarray(inputs["beta"], dtype=np.float32)

    def wpack(W):
        # [D, D] -> [128, CT, D]: wpack[p, c, e] = W[c*128+p, e]
        return np.ascontiguousarray(
            W.reshape(CT, P, D_MODEL).transpose(1, 0, 2).astype(fp8))

    def wpack_e(W):
        # [D, D] -> [128, ET, CT, 128]: [p, e, c, ce] = W[c*128+p, e*128+ce]
        return np.ascontiguousarray(
            W.reshape(CT, P, ET, P).transpose(1, 2, 0, 3).astype(fp8))

    wq_p = wpack_e(Wq)
    wk_p = wpack_e(Wkv[:, :D_MODEL])
    wv_p = wpack(Wkv[:, D_MODEL:])
    wo_p = wpack(Wo)

    in_maps = []
    for c in range(8):
        b, half = divmod(c, 2)
        hb = h[:, b, :]
        own = slice(half * QLEN, (half + 1) * QLEN)
        other = slice((1 - half) * QLEN, (2 - half) * QLEN)
        # own query-half first: keys are in core-local order, so the Q
        # projection can read the first half of hT uniformly on every core.
        hT_r = np.concatenate([hb[own, :], hb[other, :]], axis=0).T
        # [D, S] -> [128, CT, S]
        ht_p = np.ascontiguousarray(
            hT_r.reshape(CT, P, SEQ).transpose(1, 0, 2).astype(fp8))
        z_full = np.where(mask[:, b], np.float32(0.0), np.float32(1.0))
        in_maps.append({
            "ht2": ht_p,
            "hq": np.ascontiguousarray(hb[own, :]),
            "wq2": wq_p, "wk2": wk_p, "wv2": wv_p, "wo2": wo_p,
            "zt": np.ascontiguousarray(
                np.concatenate([z_full[own], z_full[other]])),
            "gam": gamma, "bet": beta,
        })
    return in_maps


def _run(in_maps, **kwargs):
    from concourse.bass_utils import run_bass_kernel_spmd
    return run_bass_kernel_spmd(_get_nc(), in_maps, core_ids=list(range(8)),
                                **kwargs)


def kernel(**inputs) -> np.ndarray:
    res = _run(_make_in_maps(inputs))
    out = np.empty((SEQ, BSZ, D_MODEL), dtype=np.float32)
    for c in range(8):
        b, half = divmod(c, 2)
        out[half * QLEN:(half + 1) * QLEN, :, :][:, b, :] = res.results[c]["out"]
    return out


# revision 30
# speedup vs baseline: 1.0308x; 1.0247x over previous
"""MultiHeadAttn (post-LN, key-padding mask) Trainium2 Bass kernel, 8 cores.

Problem: h [S=2048, B=4, D=1024] f32; 16 heads x 64; key-padding mask [S, B];
out = LayerNorm(h + MHA(h)).

Sharding: core c handles batch b = c//2 and query half qh = c%2 (1024 query
rows), with all 16 heads and the full 2048-key context for that batch.
KV projections are recomputed by both cores of a batch pair (no collectives).

Per-core device pipeline:
  - All four projections (K^T, Q^T, V, O) and the PV matmul run in fp8e4
    with perf_mode=DoubleRow: operands are [128, 2, free] slot-pair APs so
    each 512-col moving stream contracts 256 rows -> half the PE streams of
    the bf16 version.  Scores stay bf16 (K^T/Q^T psum evacuated to bf16).
  - V proj psum is evacuated by the (otherwise idle in the prephase) ACT
    engine via Copy with per-partition scale z in {0,1} (masked key -> 0);
    a z*(1/16) column is appended per head (col 64), so the DoubleRow PV
    matmul (stationary [z*V | z/16], M=65) accumulates the numerator and,
    in psum row 64, den/16.  The 1/16 keeps the fp8 avt in the normal
    e4m3 range (avt stores 16*attn_vec; the output projection's residual
    STT multiplies by 1/16).
  - Attention per head pair: scores^T [j,i] via row-paired bf16 matmuls
    (two heads in row strips 0-63 / 64-127, i-major issue order), exp via
    ScalarE with scale=1/8 and bias=-2 (softmax-invariant shift keeping
    exp values < 105 so the fp8e4 pt never overflows), written straight to
    fp8 slot j%2 of a [128, 2, 1024] pt tile.  PV consumes jtile PAIRS
    (DoubleRow contraction 256 keys) ~2 j behind the scores/exp front.
  - Normalize (two-stage software pipeline, all off the PE critical path):
    stage 1 evacuates av psum via DVE copies and computes 16/den with a
    32-lane-parallel transpose/strided-reciprocal/transpose; stage 2 -
    deferred to the NEXT pair's end - broadcasts 16/den to 64 rows on
    GPSIMD and multiplies into the fp8 avt slot (head1 via a small
    partition-shift DMA into rows 64-127).
  - Output proj: DoubleRow over e-tile pairs; residual add + LN spread
    over DVE (stt/accum), ScalarE (Square, Sqrt, gamma*rstd) and GPSIMD
    (+beta); 8 psum banks + 3-deep tile pipeline hide the chain.
Next head pair's K/Q projections are interleaved into the attention loop
(borrowing scores-pool PSUM slots) so the PE stays busy under the ACT-bound
softmax stream.
"""
import numpy as np
import ml_dtypes

N_HEAD, D_MODEL, D_HEAD = 16, 1024, 64
SEQ, BSZ = 2048, 4
QLEN = SEQ // 2
SCALE = 1.0 / D_HEAD ** 0.5
EXP_BIAS = -2.0             # softmax-invariant shift; max score 6.65 -> exp<105
LN_EPS = 1e-5
P = 128
NSL = 512                   # matmul moving-operand slab (one PSUM bank fp32)
CT = D_MODEL // P           # 8 contraction tiles
ET = D_MODEL // P           # 8 e-tiles (2 heads each)
JT = SEQ // P               # 16 key tiles
JP = JT // 2                # 8 key-tile pairs (DoubleRow PV)
JS = SEQ // NSL             # 4 key slabs
IS = QLEN // NSL            # 2 query slabs
TQ = QLEN // P              # 8 query-row tiles
HP = N_HEAD // 2            # 8 head pairs
DH1 = D_HEAD + 1            # V columns per head incl. the z (denominator) col
DHP = 80                    # padded per-head V stride (16B-aligned fp8 slots)
AVS = 16.0                  # avt stores AVS*attn_vec (fp8 range use)

_CACHE = {}


def _build():
    from contextlib import ExitStack
    import concourse.bass as bass
    import concourse.mybir as mybir
    import concourse.tile as tile
    from concourse import bacc

    dt = mybir.dt
    f32, bf16, fp8 = dt.float32, dt.bfloat16, dt.float8e4
    AF = mybir.ActivationFunctionType
    ALU = mybir.AluOpType
    DR = mybir.MatmulPerfMode.DoubleRow

    nc = bacc.Bacc(None, target_bir_lowering=False)

    # fp8 host layouts: ht2[p, c, s] = h^T[c*128+p, s]; wq2/wk2[p, e, c, ce] =
    # W[c*128+p, e*128+ce] (e-tile-major for contiguous column loads);
    # wv2/wo2[p, c, e] = W[c*128+p, e].
    ht2 = nc.dram_tensor("ht2", [P, CT, SEQ], fp8, kind="ExternalInput")
    hq = nc.dram_tensor("hq", [QLEN, D_MODEL], f32, kind="ExternalInput")
    wq2 = nc.dram_tensor("wq2", [P, ET, CT, P], fp8, kind="ExternalInput")
    wk2 = nc.dram_tensor("wk2", [P, ET, CT, P], fp8, kind="ExternalInput")
    wv2 = nc.dram_tensor("wv2", [P, CT, D_MODEL], fp8, kind="ExternalInput")
    wo2 = nc.dram_tensor("wo2", [P, CT, D_MODEL], fp8, kind="ExternalInput")
    zt = nc.dram_tensor("zt", [SEQ], f32, kind="ExternalInput")
    gam = nc.dram_tensor("gam", [D_MODEL], f32, kind="ExternalInput")
    bet = nc.dram_tensor("bet", [D_MODEL], f32, kind="ExternalInput")
    out = nc.dram_tensor("out", [QLEN, D_MODEL], f32, kind="ExternalOutput")

    with tile.TileContext(nc) as tc, ExitStack() as ctx:
        persist = ctx.enter_context(tc.tile_pool(name="persist", bufs=1))

        # K/Q tiles die after their pair's scores — roll through 2 slots
        ktq = ctx.enter_context(tc.tile_pool(name="ktq", bufs=2))
        # V in fp8 slot pairs: v2[p, slot, head, 0:64]=z*V, [.., 64]=z/16
        v_sb = [persist.tile([P, 2, N_HEAD, DHP], fp8, name=f"v{t}")
                for t in range(JP)]
        # avt in fp8 slot pairs for the DoubleRow output projection
        avt_sb = [persist.tile([P, 2, QLEN], fp8, name=f"avt{e}")
                  for e in range(ET // 2)]
        z_sb = persist.tile([P, JT], f32, name="zmask")
        ones16 = persist.tile([P, N_HEAD, 1], f32, name="ones16")
        eps_sb = persist.tile([P, 1], f32, name="eps")
        expb_sb = persist.tile([P, 1], f32, name="expb")

        nc.vector.memset(eps_sb, LN_EPS)
        nc.vector.memset(expb_sb, EXP_BIAS)
        nc.vector.memset(ones16, 1.0 / AVS)

        nc.gpsimd.dma_start(out=z_sb,
                            in_=bass.AP(tensor=zt, offset=0, ap=[[1, P], [P, JT]]))

        # ---- phase-3 weights + residual input (prefetched early) ------------
        w3p = ctx.enter_context(tc.tile_pool(name="w3p", bufs=1))
        wo_sb = w3p.tile([P, CT, D_MODEL], fp8, name="wo2")
        gam_sb = w3p.tile([P, D_MODEL], f32, name="gamr")
        bet_sb = w3p.tile([P, D_MODEL], f32, name="betr")
        hq_sb = [w3p.tile([P, D_MODEL], f32, name=f"hqr{t}") for t in range(TQ)]

        # ---- phase 1 scope: h^T residency + streamed W columns --------------
        ph1_ctx = ExitStack()
        ph1 = ph1_ctx.enter_context(tc.tile_pool(name="ph1", bufs=1))
        ht_sb = ph1.tile([P, CT, SEQ], fp8, name="ht2")

        wcol = ph1_ctx.enter_context(tc.tile_pool(name="wcol", bufs=3))

        def load_wcol(w, e, tag):
            wc = wcol.tile([P, CT, P], fp8, tag=tag, name=f"{tag}{e}")
            nc.sync.dma_start(out=wc, in_=w[:, e])
            return wc

        # startup DMA priority: the first K-proj matmul needs wkc(0) + ht,
        # so those go first; ht split over the 3 DMA-capable queues.
        wc0 = load_wcol(wk2, 0, "wkc")
        ht_engs = [nc.sync, nc.scalar, nc.gpsimd]
        for c in range(0, CT, 2):
            ht_engs[(c // 2) % 3].dma_start(out=ht_sb[:, c:c + 2, :],
                                            in_=ht2[:, c:c + 2, :])

        def kq_group(ps_ap, wc, sl):
            """4 DoubleRow matmuls: one K/Q-proj output group into psum."""
            for c in range(0, CT, 2):
                nc.tensor.matmul(ps_ap, wc[:, c:c + 2, :],
                                 ht_sb[:, c:c + 2, sl * NSL:(sl + 1) * NSL],
                                 start=(c == 0), stop=(c == CT - 2),
                                 perf_mode=DR)

        # prephase: K(0), Q(0), V (own pools, closed before attention)
        with tc.tile_pool(name="wvp", bufs=1) as wvp, \
             tc.tile_pool(name="psA", bufs=6, space="PSUM") as psA:
            wv_sb = wvp.tile([P, CT, D_MODEL], fp8, name="wv2")
            nc.scalar.dma_start(out=wv_sb, in_=wv2[:, :, :])
            wc = wc0
            kt_cur = ktq.tile([P, SEQ], bf16, tag="kt", name="kt0")
            qt_cur = ktq.tile([P, QLEN], bf16, tag="qt", name="qt0")
            for j in range(JS):
                ps = psA.tile([P, NSL], f32, tag="psa", name=f"psk0_{j}")
                kq_group(ps, wc, j)
                nc.vector.tensor_copy(kt_cur[:, j * NSL:(j + 1) * NSL], ps)
            wc = load_wcol(wq2, 0, "wqc")
            for i in range(IS):
                ps = psA.tile([P, NSL], f32, tag="psa", name=f"psq0_{i}")
                kq_group(ps, wc, i)
                nc.vector.tensor_copy(qt_cur[:, i * NSL:(i + 1) * NSL], ps)
            # V projection: stationary h^T slot pairs, moving Wv slabs.
            # The psum->sbuf copy scales V rows by the per-key mask z (so
            # masked keys contribute nothing), and the z/16 column (col 64
            # per head) makes the PV matmul accumulate den/16 in psum row 64.
            for t in range(JT):
                jp, sl8 = divmod(t, 2)
                for es in range(2):
                    ps = psA.tile([P, NSL], f32, tag="psa", name=f"psv{t}_{es}")
                    for c in range(0, CT, 2):
                        nc.tensor.matmul(
                            ps, ht_sb[:, c:c + 2, t * P:(t + 1) * P],
                            wv_sb[:, c:c + 2, es * NSL:(es + 1) * NSL],
                            start=(c == 0), stop=(c == CT - 2), perf_mode=DR)
                    nc.vector.tensor_scalar(
                        out=v_sb[jp][:, sl8, es * 8:(es + 1) * 8, 0:D_HEAD],
                        in0=ps[:, :].rearrange("p (h d) -> p h d", d=D_HEAD),
                        scalar1=z_sb[:, t:t + 1], scalar2=None,
                        op0=ALU.mult)
                nc.vector.tensor_scalar(
                    out=v_sb[jp][:, sl8, :, D_HEAD:DH1], in0=ones16,
                    scalar1=z_sb[:, t:t + 1], scalar2=None, op0=ALU.mult)

        def pv_mm(av, hp, jp, pts, hb, i):
            first, last = (jp == 0), (jp == JP - 1)
            h = hp * 2 + hb
            nc.tensor.matmul(
                av[hb][i][0:DH1, :], v_sb[jp][:, :, h, 0:DH1],
                pts[hb][:, :, i * NSL:(i + 1) * NSL],
                start=first, stop=last, perf_mode=DR,
                tile_position=(0, 0),
                skip_group_check=(hb + i > 0))

        def emit_pv(nc, v_sb, av, hp, jp, pts):
            for hb in range(2):
                for i in range(IS):
                    pv_mm(av, hp, jp, pts, hb, i)

        nc.scalar.dma_start(out=wo_sb, in_=wo2[:, :, :])
        nc.gpsimd.dma_start(out=gam_sb,
                            in_=bass.AP(tensor=gam, offset=0, ap=[[0, P], [1, D_MODEL]]))
        nc.gpsimd.dma_start(out=bet_sb,
                            in_=bass.AP(tensor=bet, offset=0, ap=[[0, P], [1, D_MODEL]]))
        hq_engs = [nc.sync, nc.scalar, nc.gpsimd]
        for t in range(TQ):
            hq_engs[t % 3].dma_start(out=hq_sb[t], in_=hq[t * P:(t + 1) * P, :])

        # ---- attention ------------------------------------------------------
        attn_ctx = ExitStack()
        scp = attn_ctx.enter_context(tc.tile_pool(name="scp", bufs=2, space="PSUM"))
        avp = attn_ctx.enter_context(tc.tile_pool(name="avp", bufs=4, space="PSUM"))
        ptp = attn_ctx.enter_context(tc.tile_pool(name="ptp", bufs=6))
        # two pairs of avc/rep are in flight at once (stage 2 of pair hp
        # runs at pair hp+1's end) — slot counts must cover both.
        nrmA = attn_ctx.enter_context(tc.tile_pool(name="nrmA", bufs=8))
        nrmB = attn_ctx.enter_context(tc.tile_pool(name="nrmB", bufs=6))
        nrmC = attn_ctx.enter_context(tc.tile_pool(name="nrmC", bufs=3))
        nrmT = attn_ctx.enter_context(tc.tile_pool(name="nrmT", bufs=2))

        # Normalize: two-stage software pipeline (see module docstring).
        def norm_stage1(hp, av):
            avcs, reps = [], []
            for i in range(IS):
                for hb in range(2):
                    avc = nrmA.tile([P, NSL], f32, tag="avc",
                                   name=f"avc{hp}_{hb}_{i}")
                    nc.vector.tensor_copy(avc[0:DH1, :], av[hb][i][0:DH1, :])
                    avcs.append(avc)
            for idx, avc in enumerate(avcs):
                rep = nrmB.tile([P, NSL], f32, tag="rep", name=f"rep{hp}_{idx}")
                # 32-lane-parallel reciprocal of den/16 -> 16/den (~1.4us)
                tmp = nrmT.tile([P, NSL], f32, tag="tmp", name=f"tm{hp}_{idx}")
                nc.vector.transpose(tmp[0:32, :], avc[64:96, :])
                nc.vector.reciprocal(
                    tmp[0:32, :].rearrange("p (b c) -> p c b", c=32)[:, 0:1, :],
                    tmp[0:32, :].rearrange("p (b c) -> p c b", c=32)[:, 0:1, :])
                nc.vector.transpose(rep[0:32, :], tmp[0:32, :])
                reps.append(rep)
            return avcs, reps

        def norm_stage2(hp, avcs, reps):
            ee, sl2 = divmod(hp, 2)
            for i in range(IS):
                for hb in range(2):
                    idx = i * 2 + hb
                    sl = slice(i * NSL, (i + 1) * NSL)
                    repl = nrmC.tile([P, NSL], f32, tag="repl",
                                    name=f"repl{hp}_{hb}_{i}")
                    nc.gpsimd.partition_broadcast(repl[0:64, :],
                                                  reps[idx][0:1, :])
                    if hb == 0:
                        nc.vector.tensor_mul(avt_sb[ee][0:64, sl2, sl],
                                             avcs[idx][0:64, :],
                                             repl[0:64, :])
                    else:
                        navt = nrmC.tile([P, NSL], fp8, tag="navt",
                                        name=f"navt{hp}_{i}")
                        nc.vector.tensor_mul(navt[0:64, :],
                                             avcs[idx][0:64, :],
                                             repl[0:64, :])
                        eng = nc.sync if i == 0 else nc.scalar
                        eng.dma_start(out=avt_sb[ee][64:P, sl2, sl],
                                      in_=navt[0:64, :])

        pending = None

        for hp in range(HP):
            av = [[avp.tile([P, NSL], f32, tag="av", name=f"av{hp}_{hb}_{i}")
                   for i in range(IS)] for hb in range(2)]
            # interleaved projection work for the NEXT head pair, borrowing
            # scores-pool psum slots: (emit_at_j: (kind, slab0))
            proj_work = {4: ("k", 0), 8: ("k", 2), 12: ("q", 0)} if hp + 1 < HP else {}
            # PV (jtile-pair jp) runs ~2 j behind the scores/exp front,
            # split 2+2 around the exps so the 4-matmul burst never starves
            # the ACT queue.
            pt_q = {}
            wc_k = wc_q = None
            kt_nxt = qt_nxt = None

            for j in range(JT):
                jp, sl2 = divmod(j, 2)
                if sl2 == 0:
                    pt_q[jp] = [ptp.tile([P, 2, QLEN], fp8, tag="pt",
                                         name=f"pt{hp}_{jp}_{hb}")
                                for hb in range(2)]
                # hb-major issue: exp(hb) queues right after its two score
                # matmuls, keeping the scores->exp->next-scores chain short.
                for hb in range(2):
                    base = hb * 64
                    sc = scp.tile([P, QLEN], f32, tag="sc",
                                  name=f"sc{hp}_{j}_{hb}")
                    for i in range(IS):
                        nc.tensor.matmul(
                            sc[:, i * NSL:(i + 1) * NSL],
                            kt_cur[base:base + 64, j * P:(j + 1) * P],
                            qt_cur[base:base + 64, i * NSL:(i + 1) * NSL],
                            start=True, stop=True, tile_position=(base, 0))
                    nc.scalar.activation(pt_q[jp][hb][:, sl2, :], sc,
                                         AF.Exp, scale=SCALE,
                                         bias=expb_sb[:, 0:1])
                if j >= 3 and j % 2 == 1:
                    emit_pv(nc, v_sb, av, hp, (j - 3) // 2, pt_q[(j - 3) // 2])

                if j in proj_work:
                    kind, sl0 = proj_work[j]
                    borrow = scp.tile([P, QLEN], f32, tag="sc",
                                      name=f"bw{hp}_{j}")
                    if kind == "k":
                        if sl0 == 0:
                            wc_k = load_wcol(wk2, hp + 1, "wkc")
                            kt_nxt = ktq.tile([P, SEQ], bf16, tag="kt",
                                              name=f"kt{hp + 1}")
                        for g in range(2):
                            sl = sl0 + g
                            kq_group(borrow[:, g * NSL:(g + 1) * NSL], wc_k, sl)
                            nc.vector.tensor_copy(
                                kt_nxt[:, sl * NSL:(sl + 1) * NSL],
                                borrow[:, g * NSL:(g + 1) * NSL])
                    else:
                        wc_q = load_wcol(wq2, hp + 1, "wqc")
                        qt_nxt = ktq.tile([P, QLEN], bf16, tag="qt",
                                          name=f"qt{hp + 1}")
                        for g in range(IS):
                            kq_group(borrow[:, g * NSL:(g + 1) * NSL], wc_q, g)
                            nc.vector.tensor_copy(
                                qt_nxt[:, g * NSL:(g + 1) * NSL],
                                borrow[:, g * NSL:(g + 1) * NSL])

            # drain the PV pipeline (jp 0..JP-2 emitted in-loop)
            emit_pv(nc, v_sb, av, hp, JP - 1, pt_q[JP - 1])

            # evacuate THIS pair's av psum first, then complete the
            # PREVIOUS pair's deferred normalize.
            cur = norm_stage1(hp, av)
            if pending is not None:
                norm_stage2(hp - 1, *pending)
            pending = cur
            if kt_nxt is not None:
                kt_cur, qt_cur = kt_nxt, qt_nxt

        norm_stage2(HP - 1, *pending)

        # ---- output projection + residual + layernorm -----------------------
        attn_ctx.close()
        ph1_ctx.close()

        pso = ctx.enter_context(tc.tile_pool(name="pso", bufs=8, space="PSUM"))
        lnp = ctx.enter_context(tc.tile_pool(name="lnp", bufs=3))
        lns = ctx.enter_context(tc.tile_pool(name="lns", bufs=8))

        for t in range(TQ):
            hq_t = hq_sb[t]
            xs = lnp.tile([P, D_MODEL], f32, tag="xs", name=f"xs{t}")
            sums = lns.tile([P, 2], f32, tag="sm", name=f"sm{t}")
            for m in range(2):
                ps = pso.tile([P, NSL], f32, tag="po", name=f"po{t}_{m}")
                for ee in range(ET // 2):
                    nc.tensor.matmul(
                        ps, avt_sb[ee][:, :, t * P:(t + 1) * P],
                        wo_sb[:, 2 * ee:2 * ee + 2, m * NSL:(m + 1) * NSL],
                        start=(ee == 0), stop=(ee == ET // 2 - 1),
                        perf_mode=DR)
                nc.vector.scalar_tensor_tensor(
                    out=xs[:, m * NSL:(m + 1) * NSL], in0=ps, scalar=1.0 / AVS,
                    in1=hq_t[:, m * NSL:(m + 1) * NSL],
                    op0=ALU.mult, op1=ALU.add,
                    accum_out=sums[:, m:m + 1])
            # mean/var via accum sums + ACT Square pass:
            # mean = (s0+s1)/D; var = sq/D - mean^2
            sq = lns.tile([P, 2], f32, tag="sq", name=f"sq{t}")
            xsq = lnp.tile([P, D_MODEL], f32, tag="xq", name=f"xq{t}")
            for m in range(2):
                nc.scalar.activation(xsq[:, m * NSL:(m + 1) * NSL],
                                     xs[:, m * NSL:(m + 1) * NSL], AF.Square,
                                     accum_out=sq[:, m:m + 1])
            mean = lns.tile([P, 1], f32, tag="mn", name=f"mn{t}")
            nc.vector.tensor_add(mean, sums[:, 0:1], sums[:, 1:2])
            nc.vector.tensor_scalar_mul(mean, mean, 1.0 / D_MODEL)
            msq = lns.tile([P, 1], f32, tag="mq", name=f"mq{t}")
            nc.vector.tensor_mul(msq, mean, mean)
            var = lns.tile([P, 1], f32, tag="vr", name=f"vr{t}")
            nc.vector.tensor_add(var, sq[:, 0:1], sq[:, 1:2])
            nc.vector.scalar_tensor_tensor(
                out=var, in0=var, scalar=1.0 / D_MODEL, in1=msq,
                op0=ALU.mult, op1=ALU.subtract)
            std = lns.tile([P, 1], f32, tag="sd", name=f"sd{t}")
            nc.scalar.activation(std, var, AF.Sqrt, bias=eps_sb[:, 0:1])
            rstd = lns.tile([P, 1], f32, tag="rs", name=f"rs{t}")
            nc.vector.reciprocal(rstd, std)
            nmr = lns.tile([P, 1], f32, tag="nm", name=f"nm{t}")
            nc.vector.tensor_scalar_mul(nmr, mean, -1.0)
            # gs on ACT (Copy with per-partition scale), xn on gpsimd
            gs = lnp.tile([P, D_MODEL], f32, tag="gs", name=f"gs{t}")
            nc.scalar.activation(gs, gam_sb, AF.Copy, scale=rstd[:, 0:1])
            xg = lnp.tile([P, D_MODEL], f32, tag="xg", name=f"xg{t}")
            nc.vector.scalar_tensor_tensor(
                out=xg, in0=xs, scalar=nmr[:, 0:1], in1=gs,
                op0=ALU.add, op1=ALU.mult)
            xn = lnp.tile([P, D_MODEL], f32, tag="xn", name=f"xn{t}")
            nc.gpsimd.tensor_add(xn, xg, bet_sb)
            nc.sync.dma_start(out=out[t * P:(t + 1) * P, :], in_=xn)

    nc.compile()
    return nc


def _get_nc():
    if "nc" not in _CACHE:
        _CACHE["nc"] = _build()
    return _CACHE["nc"]


def _make_in_maps(inputs):
    fp8 = ml_dtypes.float8_e4m3
    h = np.asarray(inputs["h"], dtype=np.float32)
    mask = np.asarray(inputs["attn_mask"])
    Wq = np.asarray(inputs["Wq"], dtype=np.float32)
    Wkv = np.asarray(inputs["Wkv"], dtype=np.float32)
    Wo = np.asarray(inputs["Wo"], dtype=np.float32)
    gamma = np.asarray(inputs["gamma"], dtype=np.float32)
    beta = np.asarray(inputs["beta"], dtype=np.float32)

    def wpack(W):
        # [D, D] -> [128, CT, D]: wpack[p, c, e] = W[c*128+p, e]
        return np.ascontiguousarray(
            W.reshape(CT, P, D_MODEL).transpose(1, 0, 2).astype(fp8))

    def wpack_e(W):
        # [D, D] -> [128, ET, CT, 128]: [p, e, c, ce] = W[c*128+p, e*128+ce]
        return np.ascontiguousarray(
            W.reshape(CT, P, ET, P).transpose(1, 2, 0, 3).astype(fp8))

    wq_p = wpack_e(Wq)
    wk_p = wpack_e(Wkv[:, :D_MODEL])
    wv_p = wpack(Wkv[:, D_MODEL:])
    wo_p = wpack(Wo)

    in_maps = []
    for c in range(8):
        b, half = divmod(c, 2)
        hb = h[:, b, :]
        own = slice(half * QLEN, (half + 1) * QLEN)
        other = slice((1 - half) * QLEN, (2 - half) * QLEN)
        # own query-half first: keys are in core-local order, so the Q
        # projection can read the first half of hT uniformly on every core.
        hT_r = np.concatenate([hb[own, :], hb[other, :]], axis=0).T
        # [D, S] -> [128, CT, S]
        ht_p = np.ascontiguousarray(
            hT_r.reshape(CT, P, SEQ).transpose(1, 0, 2).astype(fp8))
        z_full = np.where(mask[:, b], np.float32(0.0), np.float32(1.0))
        in_maps.append({
            "ht2": ht_p,
            "hq": np.ascontiguousarray(hb[own, :]),
            "wq2": wq_p, "wk2": wk_p, "wv2": wv_p, "wo2": wo_p,
            "zt": np.ascontiguousarray(
                np.concatenate([z_full[own], z_full[other]])),
            "gam": gamma, "bet": beta,
        })
    return in_maps


def _run(in_maps, **kwargs):
    from concourse.bass_utils import run_bass_kernel_spmd
    return run_bass_kernel_spmd(_get_nc(), in_maps, core_ids=list(range(8)),
                                **kwargs)


def kernel(**inputs) -> np.ndarray:
    res = _run(_make_in_maps(inputs))
    out = np.empty((SEQ, BSZ, D_MODEL), dtype=np.float32)
    for c in range(8):
        b, half = divmod(c, 2)
        out[half * QLEN:(half + 1) * QLEN, :, :][:, b, :] = res.results[c]["out"]
    return out
